# revision 18
# baseline (speedup 1.0000x reference)
"""Banded (sliding-window) multi-head attention on 8 Trainium2 NeuronCores.

Problem: B=2, S=2048, D=512, H=8 heads (hd=64), window=256 (|i-j| <= 128),
  qkv = x @ Wqkv + bqkv           -> per-head q,k,v
  scores = (q k^T masked to band) / 8 ; softmax ; out = (attn v) @ Wo + bo

Sharding: core = (batch b in {0,1}) x (head-group g in {0..3}); each core
computes 2 heads over the full sequence of one batch element plus the o_proj
partial product for its heads' embed slice. Host sums the 4 partials per batch
(bo is pre-divided by 4 so the partial sums reproduce the bias exactly... no:
bo/4 added on each of the 4 cores sums to bo).

Device-side layout notes (all matmuls in float32r = full-rate fp32):
  - qkv projection is computed TRANSPOSED: qkvT[fo, s] with the 384 output
    features permuted to [q0|q1|k0|k1|v0|v1] (64 cols each), so Q^T/K^T/V^T
    per head live at partition offsets {0,64} directly.
  - scores are computed key-major per 128-key block kb against the 1-3
    query blocks within the band window: ST[key, query] in PSUM, then
    ACT: P = exp(ST/8 + kmask[key]) and DVE: P *= trimask (band edges).
  - V^T is re-transposed to natural [key, hd] via the PE, augmented with a
    ones column so the attention-weight row sums (softmax denominators)
    drop out of the same AV matmul (row 64 of the [65, q] output).
  - AV accumulates per 512-query chunk over <=6 key blocks into one PSUM
    bank using the per-element has_written semantics (first matmul
    start=True clears the bank; later ones overwrite-or-accumulate).
  - o_proj: outT[fo, s] = Wo_g^T valsT (+ bo/4), DMA'd out transposed;
    host re-transposes and sums the 4 partials per batch.
"""

import os

import numpy as np

import concourse.bass as bass  # noqa: F401  (engine types via nc)
import concourse.mybir as mybir
import concourse.tile as tile
from concourse import bacc
from concourse.bass_utils import run_bass_kernel_spmd

B, S, DIN, E = 2, 2048, 512, 512
H, HD = 8, 64
NB = S // 128      # 16 key/query blocks of 128
NCHUNK = S // 512  # 4 query chunks of 512
F32 = mybir.dt.float32
F32R = mybir.dt.float32r
EXPF = mybir.ActivationFunctionType.Exp

_CACHE = {}
LAST_RESULTS = None  # BassKernelResults of the most recent run (for test.py)


def _build_nc():
    nc = bacc.Bacc(None, target_bir_lowering=False, debug=False)

    xt = nc.dram_tensor("xt", [4, DIN, 512], F32R, kind="ExternalInput")
    wq = nc.dram_tensor("wq", [128, 4, 384], F32R, kind="ExternalInput")
    wo = nc.dram_tensor("wo", [128, E], F32R, kind="ExternalInput")
    km = nc.dram_tensor("km", [128, NB], F32, kind="ExternalInput")
    tm = nc.dram_tensor("tm", [128, 384], F32, kind="ExternalInput")
    bo4 = nc.dram_tensor("bo4", [128, 4], F32, kind="ExternalInput")
    idin = nc.dram_tensor("idin", [128, 128], F32R, kind="ExternalInput")
    outt = nc.dram_tensor("outt", [E, S], F32, kind="ExternalOutput")

    with tile.TileContext(nc) as tc:
        with (
            tc.tile_pool(name="sb", bufs=1) as sb,
            tc.tile_pool(name="ps_qkv", bufs=2, space="PSUM") as ps_qkv,
            tc.tile_pool(name="ps_st", bufs=3, space="PSUM") as ps_st,
            tc.tile_pool(name="ps_ot", bufs=3, space="PSUM") as ps_ot,
            tc.tile_pool(name="small", bufs=4) as small,
        ):
            xt_sb = sb.tile([128, 4, 4, 512], F32R)   # [p, kchunk, qchunk, q]
            wq_sb = sb.tile([128, 4, 384], F32R)      # [p, kchunk, fo]
            wo_sb = sb.tile([128, E], F32R)
            km_sb = sb.tile([128, NB], F32)
            tm2_sb = sb.tile([128, 384], F32)
            bo_sb = sb.tile([128, 4], F32)
            qkvt = sb.tile([128, 3, S], F32R)         # fb0=Q, fb1=K, fb2=V (h0|h1)
            vnat = sb.tile([128, NB, 130], F32R)      # [v0|1|v1|1] per key block
            valst = sb.tile([128, S], F32R)           # normalized attn @ V, d-major
            outt_sb = sb.tile([128, 4, S], F32)
            ident = sb.tile([128, 128], F32R)

            # weights/constants on the ACT HWDGE ring, xt on the SP ring;
            # both split by k-chunk so the first qkv group starts early
            for kc in range(4):
                nc.scalar.dma_start(out=wq_sb[:, kc, :], in_=wq[:, kc, :])
                nc.sync.dma_start(
                    out=xt_sb[:, kc, 0, :],
                    in_=xt[0, kc * 128:(kc + 1) * 128, :],
                )
            for cc in range(1, 4):
                nc.sync.dma_start(
                    out=xt_sb[:, :, cc, :],
                    in_=xt[cc].rearrange("(kc p) q -> p kc q", p=128),
                )
            nc.scalar.dma_start(out=km_sb, in_=km[:, :])
            nc.scalar.dma_start(out=tm2_sb, in_=tm[:, :])
            nc.scalar.dma_start(out=ident, in_=idin[:, :])
            nc.scalar.dma_start(out=wo_sb, in_=wo[:, :])
            nc.scalar.dma_start(out=bo_sb, in_=bo4[:, :])

            # ones columns for the AV denominator rows; sourced from the
            # all-ones center block of the trimask (memset cannot write f32r)
            nc.vector.tensor_copy(vnat[:, :, 64:65], tm2_sb[:, 128:144])
            nc.vector.tensor_copy(vnat[:, :, 129:130], tm2_sb[:, 144:160])

            # ---- qkv projection (transposed): qkvT = Wg^T @ x[b]^T ----
            for cc in range(4):
                for fb in range(3):
                    ps = ps_qkv.tile([128, 512], F32, tag="qkv")
                    for kc in range(4):
                        nc.tensor.matmul(
                            ps,
                            wq_sb[:, kc, fb * 128:(fb + 1) * 128],
                            xt_sb[:, kc, cc, :],
                            start=(kc == 0),
                            stop=(kc == 3),
                        )
                    ceng = nc.scalar if (cc + fb) % 2 == 0 else nc.vector
                    if ceng is nc.scalar:
                        ceng.copy(out=qkvt[:, fb, cc * 512:(cc + 1) * 512], in_=ps)
                    else:
                        ceng.tensor_copy(qkvt[:, fb, cc * 512:(cc + 1) * 512], ps)

            # ---- V^T -> V natural [key, hd], with ones columns at 64/129 ----
            for kb in range(NB):
                pst = ps_misc.tile([128, 128], F32R, tag="misc", name="pst")
                nc.tensor.transpose(
                    pst, qkvt[:, 2, kb * 128:(kb + 1) * 128], ident
                )
                nc.vector.tensor_copy(
                    vnat[:, kb, :].rearrange("p (g c) -> p g c", c=65)[:, :, 0:64],
                    pst.rearrange("p (g c) -> p g c", c=64),
                )

            # ---- attention: heads interleaved per key block ----
            p_sb = [sb.tile([128, NB, 384], F32R, name=f"p{h}") for h in range(2)]

            def scores_block(h, kb):
                hp = 64 * h
                ws, we = max(0, kb - 1), min(NB - 1, kb + 1)
                nq = (we - ws + 1) * 128
                moff = (1 - (kb - ws)) * 128
                st = ps_st.tile([128, 384], F32, tag="st", name="st")
                nc.tensor.matmul(
                    st[:, :nq],
                    qkvt[hp:hp + 64, 1, kb * 128:(kb + 1) * 128],
                    qkvt[hp:hp + 64, 0, ws * 128:(we + 1) * 128],
                    start=True,
                    stop=True,
                )
                nc.scalar.activation(
                    out=p_sb[h][:, kb, 0:nq],
                    in_=st[:, :nq],
                    func=EXPF,
                    bias=km_sb[:, kb:kb + 1],
                    scale=0.125,
                )
                eng = nc.gpsimd if kb % 3 == 2 else nc.vector
                eng.tensor_mul(
                    p_sb[h][:, kb, 0:nq],
                    p_sb[h][:, kb, 0:nq],
                    tm2_sb[:, moff:moff + nq],
                )

            def av_chunk(h, c):
                hp = 64 * h
                kbs = list(range(max(0, 4 * c - 1), min(NB - 1, 4 * c + 4) + 1))
                ot = ps_ot.tile([65, 512], F32, tag="ot", name="ot")
                for i, kb in enumerate(kbs):
                    ws, we = max(0, kb - 1), min(NB - 1, kb + 1)
                    qs, qe = max(ws, 4 * c), min(we, 4 * c + 3)
                    nc.tensor.matmul(
                        ot[:, (qs - 4 * c) * 128:(qe + 1 - 4 * c) * 128],
                        vnat[:, kb, 65 * h:65 * h + 65],
                        p_sb[h][:, kb, (qs - ws) * 128:(qe + 1 - ws) * 128],
                        start=(i == 0),
                        stop=(i == len(kbs) - 1),
                        skip_group_check=True,
                    )
                rc = small.tile([1, 512], F32, tag="rc", name="rc")
                nc.vector.reciprocal(rc, ot[64:65, :])
                rb = small.tile([64, 512], F32, tag="rb", name="rb")
                nc.gpsimd.partition_broadcast(rb, rc)
                nc.vector.tensor_mul(
                    valst[hp:hp + 64, c * 512:(c + 1) * 512],
                    ot[0:64, :],
                    rb,
                )

            def oproj_chunk(c):
                for fo in range(4):
                    po = ps_misc.tile([128, 512], F32, tag="misc", name="po")
                    nc.tensor.matmul(
                        po,
                        wo_sb[:, fo * 128:(fo + 1) * 128],
                        valst[:, c * 512:(c + 1) * 512],
                        start=True,
                        stop=True,
                    )
                    if fo % 2 == 0:
                        nc.scalar.activation(
                            out=outt_sb[:, fo, c * 512:(c + 1) * 512],
                            in_=po,
                            func=mybir.ActivationFunctionType.Identity,
                            bias=bo_sb[:, fo:fo + 1],
                        )
                    else:
                        nc.vector.tensor_scalar_add(
                            out=outt_sb[:, fo, c * 512:(c + 1) * 512],
                            in0=po,
                            scalar1=bo_sb[:, fo:fo + 1],
                        )
                    deng = nc.sync if fo % 2 == 0 else nc.gpsimd
                    deng.dma_start(
                        out=outt[fo * 128:(fo + 1) * 128, c * 512:(c + 1) * 512],
                        in_=outt_sb[:, fo, c * 512:(c + 1) * 512],
                    )

            for kb in range(NB):
                for h in range(2):
                    scores_block(h, kb)
                # chunk c's AV window ends at kb = 4c+4 (or NB-1 for the last)
                if kb >= 4 and kb % 4 == 0:
                    c = kb // 4 - 1
                    for h in range(2):
                        av_chunk(h, c)
                    oproj_chunk(c)
            for h in range(2):
                av_chunk(h, NCHUNK - 1)
            oproj_chunk(NCHUNK - 1)

    nc.finalize()
    return nc


def _numpy_reference(x, padding_mask, Wqkv, bqkv, Wo, bo):
    """Fallback for input regimes the device path does not cover."""
    b, s, _ = x.shape
    qkv = x @ Wqkv + bqkv
    qkv = qkv.reshape(b, s, H, 3 * HD).transpose(0, 2, 1, 3)
    q, k, v = np.split(qkv, 3, axis=-1)
    scores = np.einsum("bhqd,bhkd->bhqk", q, k)
    idx = np.arange(s)
    band = np.abs(idx[:, None] - idx[None, :]) <= 128
    pm = padding_mask != 0
    valid = band[None, None] & pm[:, None, None, :] & pm[:, None, :, None]
    scores = np.where(valid, scores, -np.inf) / np.sqrt(HD)
    scores = scores - scores.max(axis=-1, keepdims=True)
    with np.errstate(invalid="ignore", over="ignore"):
        e = np.exp(scores)
        attn = e / e.sum(axis=-1, keepdims=True)
    attn = np.nan_to_num(attn, nan=0.0)
    vals = np.einsum("bhqk,bhkd->bhqd", attn, v)
    vals = vals.transpose(0, 2, 1, 3).reshape(b, s, E)
    return (vals @ Wo + bo).astype(np.float32)


def kernel(x, padding_mask, Wqkv, bqkv, Wo, bo):
    global LAST_RESULTS
    x = np.ascontiguousarray(np.asarray(x, np.float32))
    Wqkv = np.asarray(Wqkv, np.float32)
    bqkv = np.asarray(bqkv, np.float32)
    Wo = np.asarray(Wo, np.float32)
    bo = np.asarray(bo, np.float32)
    pm = np.asarray(padding_mask)

    if np.any(bqkv != 0):
        # qkv bias is identically zero in the target problem; the device
        # program folds no qkv bias, so fall back rather than be wrong.
        return _numpy_reference(x, pm, Wqkv, bqkv, Wo, bo)

    if "nc" not in _CACHE:
        _CACHE["nc"] = _build_nc()
    nc = _CACHE["nc"]

    # trimask [key p, 384]: window cols = [qb-1 | qb | qb+1] relative blocks
    j = np.arange(128)[:, None]
    i = np.arange(128)[None, :]
    tm = np.concatenate(
        [(j <= i), np.ones((128, 128), bool), (j >= i)], axis=1
    ).astype(np.float32)

    in_maps = []
    for core in range(8):
        b, g = divmod(core, 4)
        # feature permutation for this head group: [q0|q1|k0|k1|v0|v1]
        h0, h1 = 2 * g, 2 * g + 1
        cols = []
        for kind in range(3):  # q, k, v
            for h in (h0, h1):
                base = h * 3 * HD + kind * HD
                cols.extend(range(base, base + HD))
        wq_g = Wqkv[:, cols]                                  # [512, 384]
        xt_b = np.ascontiguousarray(x[b].T)                   # [512, 2048]
        xt_cc = np.stack([xt_b[:, cc * 512:(cc + 1) * 512] for cc in range(4)])
        km = np.where(pm[b] != 0, 0.0, -1e5).astype(np.float32)
        in_maps.append({
            "xt": np.ascontiguousarray(xt_cc, dtype=np.float32),
            "wq": np.ascontiguousarray(
                wq_g.reshape(4, 128, 384).transpose(1, 0, 2), dtype=np.float32),
            "wo": np.ascontiguousarray(
                Wo[g * 128:(g + 1) * 128, :], dtype=np.float32),
            "km": np.ascontiguousarray(km.reshape(NB, 128).T, dtype=np.float32),
            "tm": tm,
            "bo4": np.ascontiguousarray(
                (bo / 4.0).reshape(4, 128).T, dtype=np.float32),
            "idin": np.eye(128, dtype=np.float32),
        })

    trace = os.environ.get("KERNEL_TRACE", "0") == "1"
    kwargs = {}
    if trace:
        kwargs = dict(trace=True, trace_cores=[0], stitch_traces=False)
    LAST_RESULTS = run_bass_kernel_spmd(
        nc, in_maps, core_ids=list(range(8)), **kwargs
    )
    res = LAST_RESULTS.results

    out = np.zeros((B, S, E), np.float32)
    for core in range(8):
        b = core // 4
        out[b] += res[core]["outt"].T
    return out


# revision 19
# speedup vs baseline: 1.0021x; 1.0021x over previous
"""Banded (sliding-window) multi-head attention on 8 Trainium2 NeuronCores.

Problem: B=2, S=2048, D=512, H=8 heads (hd=64), window=256 (|i-j| <= 128),
  qkv = x @ Wqkv + bqkv           -> per-head q,k,v
  scores = (q k^T masked to band) / 8 ; softmax ; out = (attn v) @ Wo + bo

Sharding: core = (batch b in {0,1}) x (head-group g in {0..3}); each core
computes 2 heads over the full sequence of one batch element plus the o_proj
partial product for its heads' embed slice. Host sums the 4 partials per batch
(bo is pre-divided by 4 so the partial sums reproduce the bias exactly... no:
bo/4 added on each of the 4 cores sums to bo).

Device-side layout notes (all matmuls in float32r = full-rate fp32):
  - qkv projection is computed TRANSPOSED: qkvT[fo, s] with the 384 output
    features permuted to [q0|q1|k0|k1|v0|v1] (64 cols each), so Q^T/K^T/V^T
    per head live at partition offsets {0,64} directly.
  - scores are computed key-major per 128-key block kb against the 1-3
    query blocks within the band window: ST[key, query] in PSUM, then
    ACT: P = exp(ST/8 + kmask[key]) and DVE: P *= trimask (band edges).
  - V^T is re-transposed to natural [key, hd] via the PE, augmented with a
    ones column so the attention-weight row sums (softmax denominators)
    drop out of the same AV matmul (row 64 of the [65, q] output).
  - AV accumulates per 512-query chunk over <=6 key blocks into one PSUM
    bank using the per-element has_written semantics (first matmul
    start=True clears the bank; later ones overwrite-or-accumulate).
  - o_proj: outT[fo, s] = Wo_g^T valsT (+ bo/4), DMA'd out transposed;
    host re-transposes and sums the 4 partials per batch.
"""

import os

import numpy as np

import concourse.bass as bass  # noqa: F401  (engine types via nc)
import concourse.mybir as mybir
import concourse.tile as tile
from concourse import bacc
from concourse.bass_utils import run_bass_kernel_spmd

B, S, DIN, E = 2, 2048, 512, 512
H, HD = 8, 64
NB = S // 128      # 16 key/query blocks of 128
NCHUNK = S // 512  # 4 query chunks of 512
F32 = mybir.dt.float32
F32R = mybir.dt.float32r
EXPF = mybir.ActivationFunctionType.Exp

_CACHE = {}
LAST_RESULTS = None  # BassKernelResults of the most recent run (for test.py)


def _build_nc():
    nc = bacc.Bacc(None, target_bir_lowering=False, debug=False)

    xt = nc.dram_tensor("xt", [4, DIN, 512], F32R, kind="ExternalInput")
    wq = nc.dram_tensor("wq", [128, 4, 384], F32R, kind="ExternalInput")
    wo = nc.dram_tensor("wo", [128, E], F32R, kind="ExternalInput")
    km = nc.dram_tensor("km", [128, NB], F32, kind="ExternalInput")
    tm = nc.dram_tensor("tm", [128, 384], F32, kind="ExternalInput")
    bo4 = nc.dram_tensor("bo4", [128, 4], F32, kind="ExternalInput")
    idin = nc.dram_tensor("idin", [128, 128], F32R, kind="ExternalInput")
    outt = nc.dram_tensor("outt", [E, S], F32, kind="ExternalOutput")

    with tile.TileContext(nc) as tc:
        with (
            tc.tile_pool(name="sb", bufs=1) as sb,
            tc.tile_pool(name="ps_qkv", bufs=2, space="PSUM") as ps_qkv,
            tc.tile_pool(name="ps_st", bufs=3, space="PSUM") as ps_st,
            tc.tile_pool(name="ps_ot", bufs=3, space="PSUM") as ps_ot,
            tc.tile_pool(name="small", bufs=4) as small,
        ):
            xt_sb = sb.tile([128, 4, 4, 512], F32R)   # [p, kchunk, qchunk, q]
            wq_sb = sb.tile([128, 4, 384], F32R)      # [p, kchunk, fo]
            wo_sb = sb.tile([128, E], F32R)
            km_sb = sb.tile([128, NB], F32)
            tm2_sb = sb.tile([128, 384], F32)
            bo_sb = sb.tile([128, 4], F32)
            qkvt = sb.tile([128, 3, S], F32R)         # fb0=Q, fb1=K, fb2=V (h0|h1)
            vnat = sb.tile([128, NB, 130], F32R)      # [v0|1|v1|1] per key block
            valst = sb.tile([128, S], F32R)           # normalized attn @ V, d-major
            outt_sb = sb.tile([128, 4, S], F32)
            ident = sb.tile([128, 128], F32R)

            # weights/constants on the ACT HWDGE ring, xt on the SP ring;
            # both split by k-chunk so the first qkv group starts early
            for kc in range(4):
                nc.scalar.dma_start(out=wq_sb[:, kc, :], in_=wq[:, kc, :])
                nc.sync.dma_start(
                    out=xt_sb[:, kc, 0, :],
                    in_=xt[0, kc * 128:(kc + 1) * 128, :],
                )
            for cc in range(1, 4):
                nc.sync.dma_start(
                    out=xt_sb[:, :, cc, :],
                    in_=xt[cc].rearrange("(kc p) q -> p kc q", p=128),
                )
            nc.scalar.dma_start(out=km_sb, in_=km[:, :])
            nc.scalar.dma_start(out=tm2_sb, in_=tm[:, :])
            nc.scalar.dma_start(out=ident, in_=idin[:, :])
            nc.scalar.dma_start(out=wo_sb, in_=wo[:, :])
            nc.scalar.dma_start(out=bo_sb, in_=bo4[:, :])

            # ones columns for the AV denominator rows; sourced from the
            # all-ones center block of the trimask (memset cannot write f32r)
            nc.vector.tensor_copy(vnat[:, :, 64:65], tm2_sb[:, 128:144])
            nc.vector.tensor_copy(vnat[:, :, 129:130], tm2_sb[:, 144:160])

            # ---- qkv projection (transposed): qkvT = Wg^T @ x[b]^T ----
            for cc in range(4):
                for fb in range(3):
                    ps = ps_qkv.tile([128, 512], F32, tag="qkv")
                    for kc in range(4):
                        nc.tensor.matmul(
                            ps,
                            wq_sb[:, kc, fb * 128:(fb + 1) * 128],
                            xt_sb[:, kc, cc, :],
                            start=(kc == 0),
                            stop=(kc == 3),
                        )
                    ceng = nc.scalar if (cc + fb) % 2 == 0 else nc.vector
                    if ceng is nc.scalar:
                        ceng.copy(out=qkvt[:, fb, cc * 512:(cc + 1) * 512], in_=ps)
                    else:
                        ceng.tensor_copy(qkvt[:, fb, cc * 512:(cc + 1) * 512], ps)

            # ---- V^T -> V natural [key, hd], with ones columns at 64/129 ----
            for kb in range(NB):
                pst = ps_misc.tile([128, 128], F32R, tag="misc", name="pst")
                nc.tensor.transpose(
                    pst, qkvt[:, 2, kb * 128:(kb + 1) * 128], ident
                )
                nc.vector.tensor_copy(
                    vnat[:, kb, :].rearrange("p (g c) -> p g c", c=65)[:, :, 0:64],
                    pst.rearrange("p (g c) -> p g c", c=64),
                )

            # ---- attention: heads interleaved per key block ----
            p_sb = [sb.tile([128, NB, 384], F32R, name=f"p{h}") for h in range(2)]

            def scores_block(h, kb):
                hp = 64 * h
                ws, we = max(0, kb - 1), min(NB - 1, kb + 1)
                nq = (we - ws + 1) * 128
                moff = (1 - (kb - ws)) * 128
                st = ps_st.tile([128, 384], F32, tag="st", name="st")
                nc.tensor.matmul(
                    st[:, :nq],
                    qkvt[hp:hp + 64, 1, kb * 128:(kb + 1) * 128],
                    qkvt[hp:hp + 64, 0, ws * 128:(we + 1) * 128],
                    start=True,
                    stop=True,
                )
                nc.scalar.activation(
                    out=p_sb[h][:, kb, 0:nq],
                    in_=st[:, :nq],
                    func=EXPF,
                    bias=km_sb[:, kb:kb + 1],
                    scale=0.125,
                )
                eng = nc.gpsimd if kb % 3 == 2 else nc.vector
                eng.tensor_mul(
                    p_sb[h][:, kb, 0:nq],
                    p_sb[h][:, kb, 0:nq],
                    tm2_sb[:, moff:moff + nq],
                )

            def av_chunk(h, c):
                hp = 64 * h
                kbs = list(range(max(0, 4 * c - 1), min(NB - 1, 4 * c + 4) + 1))
                ot = ps_ot.tile([65, 512], F32, tag="ot", name="ot")
                for i, kb in enumerate(kbs):
                    ws, we = max(0, kb - 1), min(NB - 1, kb + 1)
                    qs, qe = max(ws, 4 * c), min(we, 4 * c + 3)
                    nc.tensor.matmul(
                        ot[:, (qs - 4 * c) * 128:(qe + 1 - 4 * c) * 128],
                        vnat[:, kb, 65 * h:65 * h + 65],
                        p_sb[h][:, kb, (qs - ws) * 128:(qe + 1 - ws) * 128],
                        start=(i == 0),
                        stop=(i == len(kbs) - 1),
                        skip_group_check=True,
                    )
                rc = small.tile([1, 512], F32, tag="rc", name="rc")
                nc.vector.reciprocal(rc, ot[64:65, :])
                rb = small.tile([64, 512], F32, tag="rb", name="rb")
                nc.gpsimd.partition_broadcast(rb, rc)
                nc.vector.tensor_mul(
                    valst[hp:hp + 64, c * 512:(c + 1) * 512],
                    ot[0:64, :],
                    rb,
                )

            def oproj_chunk(c):
                for fo in range(4):
                    po = (ps_misc if fo % 2 == 0 else ps_ot).tile(
                        [128, 512], F32, tag="misc" if fo % 2 == 0 else "ot",
                        name="po")
                    nc.tensor.matmul(
                        po,
                        wo_sb[:, fo * 128:(fo + 1) * 128],
                        valst[:, c * 512:(c + 1) * 512],
                        start=True,
                        stop=True,
                    )
                    if fo % 2 == 0:
                        nc.scalar.activation(
                            out=outt_sb[:, fo, c * 512:(c + 1) * 512],
                            in_=po,
                            func=mybir.ActivationFunctionType.Identity,
                            bias=bo_sb[:, fo:fo + 1],
                        )
                    else:
                        nc.vector.tensor_scalar_add(
                            out=outt_sb[:, fo, c * 512:(c + 1) * 512],
                            in0=po,
                            scalar1=bo_sb[:, fo:fo + 1],
                        )
                    deng = nc.sync if fo % 2 == 0 else nc.gpsimd
                    deng.dma_start(
                        out=outt[fo * 128:(fo + 1) * 128, c * 512:(c + 1) * 512],
                        in_=outt_sb[:, fo, c * 512:(c + 1) * 512],
                    )

            for kb in range(NB):
                for h in range(2):
                    scores_block(h, kb)
                # chunk c's AV window ends at kb = 4c+4 (or NB-1 for the last)
                if kb >= 4 and kb % 4 == 0:
                    c = kb // 4 - 1
                    for h in range(2):
                        av_chunk(h, c)
                    oproj_chunk(c)
            for h in range(2):
                av_chunk(h, NCHUNK - 1)
            oproj_chunk(NCHUNK - 1)

    nc.finalize()
    return nc


def _numpy_reference(x, padding_mask, Wqkv, bqkv, Wo, bo):
    """Fallback for input regimes the device path does not cover."""
    b, s, _ = x.shape
    qkv = x @ Wqkv + bqkv
    qkv = qkv.reshape(b, s, H, 3 * HD).transpose(0, 2, 1, 3)
    q, k, v = np.split(qkv, 3, axis=-1)
    scores = np.einsum("bhqd,bhkd->bhqk", q, k)
    idx = np.arange(s)
    band = np.abs(idx[:, None] - idx[None, :]) <= 128
    pm = padding_mask != 0
    valid = band[None, None] & pm[:, None, None, :] & pm[:, None, :, None]
    scores = np.where(valid, scores, -np.inf) / np.sqrt(HD)
    scores = scores - scores.max(axis=-1, keepdims=True)
    with np.errstate(invalid="ignore", over="ignore"):
        e = np.exp(scores)
        attn = e / e.sum(axis=-1, keepdims=True)
    attn = np.nan_to_num(attn, nan=0.0)
    vals = np.einsum("bhqk,bhkd->bhqd", attn, v)
    vals = vals.transpose(0, 2, 1, 3).reshape(b, s, E)
    return (vals @ Wo + bo).astype(np.float32)


def kernel(x, padding_mask, Wqkv, bqkv, Wo, bo):
    global LAST_RESULTS
    x = np.ascontiguousarray(np.asarray(x, np.float32))
    Wqkv = np.asarray(Wqkv, np.float32)
    bqkv = np.asarray(bqkv, np.float32)
    Wo = np.asarray(Wo, np.float32)
    bo = np.asarray(bo, np.float32)
    pm = np.asarray(padding_mask)

    if np.any(bqkv != 0):
        # qkv bias is identically zero in the target problem; the device
        # program folds no qkv bias, so fall back rather than be wrong.
        return _numpy_reference(x, pm, Wqkv, bqkv, Wo, bo)

    if "nc" not in _CACHE:
        _CACHE["nc"] = _build_nc()
    nc = _CACHE["nc"]

    # trimask [key p, 384]: window cols = [qb-1 | qb | qb+1] relative blocks
    j = np.arange(128)[:, None]
    i = np.arange(128)[None, :]
    tm = np.concatenate(
        [(j <= i), np.ones((128, 128), bool), (j >= i)], axis=1
    ).astype(np.float32)

    in_maps = []
    for core in range(8):
        b, g = divmod(core, 4)
        # feature permutation for this head group: [q0|q1|k0|k1|v0|v1]
        h0, h1 = 2 * g, 2 * g + 1
        cols = []
        for kind in range(3):  # q, k, v
            for h in (h0, h1):
                base = h * 3 * HD + kind * HD
                cols.extend(range(base, base + HD))
        wq_g = Wqkv[:, cols]                                  # [512, 384]
        xt_b = np.ascontiguousarray(x[b].T)                   # [512, 2048]
        xt_cc = np.stack([xt_b[:, cc * 512:(cc + 1) * 512] for cc in range(4)])
        km = np.where(pm[b] != 0, 0.0, -1e5).astype(np.float32)
        in_maps.append({
            "xt": np.ascontiguousarray(xt_cc, dtype=np.float32),
            "wq": np.ascontiguousarray(
                wq_g.reshape(4, 128, 384).transpose(1, 0, 2), dtype=np.float32),
            "wo": np.ascontiguousarray(
                Wo[g * 128:(g + 1) * 128, :], dtype=np.float32),
            "km": np.ascontiguousarray(km.reshape(NB, 128).T, dtype=np.float32),
            "tm": tm,
            "bo4": np.ascontiguousarray(
                (bo / 4.0).reshape(4, 128).T, dtype=np.float32),
            "idin": np.eye(128, dtype=np.float32),
        })

    trace = os.environ.get("KERNEL_TRACE", "0") == "1"
    kwargs = {}
    if trace:
        kwargs = dict(trace=True, trace_cores=[0], stitch_traces=False)
    LAST_RESULTS = run_bass_kernel_spmd(
        nc, in_maps, core_ids=list(range(8)), **kwargs
    )
    res = LAST_RESULTS.results

    out = np.zeros((B, S, E), np.float32)
    for core in range(8):
        b = core // 4
        out[b] += res[core]["outt"].T
    return out
